# revision 13
# baseline (speedup 1.0000x reference)
"""DynamicGNN (GCN x2 -> windowed GRU -> predictor) on 8 Trainium2 NeuronCores.

Self-contained: takes FULL inputs as numpy arrays, returns FULL [T, N] f32.

Strategy (data-parallel over T, 8 timesteps per core, two SPMD launches):

GCN launch (per core, per t):
  p1T = W1.T @ xt.T            (PE, transposed layout [64, NPAD])
  tbl1 = p1T.T -> HBM          (PE transpose + DMA; node-major gather table)
  A1.T = sparse-propagate      (dma_gather of 128-edge chunks + PE matmuls
                                against host-packed "slabs" that bake in the
                                full GCN norm dis[r]*w*dis[c]; chunk matmul:
                                psum[64, cols] = msgs.T @ slab)
  h1T = relu(A1.T + b1)        (ACT eviction, bias fused)
  ... same again with W2/b2 -> gcnT [64, TPC*NPAD] -> HBM

GRU launch (per core): batched over the 8 windows; the [T,W,N,64] window
tensor never exists -- the sliding window is a column offset into the
gcn halo [64, 15*NPAD]. Early-t invalid steps are made exact no-ops via
host-crafted sentinel columns that force the z gate to 1.
  per step s, per 512-col slice: psum[128,512] = lhsA.T@gcn + lhsB.T@h
  (gate layout [rz|in|hn]); sigmoid/tanh on ACT, elementwise on DVE.
  preds = Wp.T @ hT + bp.
"""

import numpy as np

import concourse.bass as bass
import concourse.bacc as bacc
import concourse.mybir as mybir
import concourse.tile as tile
from concourse.bass_utils import run_bass_kernel_spmd
from concourse.masks import make_identity

F32 = mybir.dt.float32
I16 = mybir.dt.int16

T, F, N, E = 64, 32, 3000, 100000
H1, H2 = 64, 32
NCORE = 8
TPC = T // NCORE
NPAD = 3072
WWIN = 8
DUMP = NPAD - 1
GATHER_GROUP = 64           # chunks per dma_gather instruction
HALO = WWIN - 1 + TPC       # 15 timesteps of gcn output per core
RGRU = TPC * NPAD           # gru batch-row count (as columns)
NSLICE = RGRU // 512        # 48

_PROG_CACHE = {}


# ---------------------------------------------------------------- host side

def build_templates_and_pack(edge_index, edge_weight):
    ei = np.asarray(edge_index)
    ew = np.asarray(edge_weight, np.float64)

    per_ct = [[None] * TPC for _ in range(NCORE)]
    for k in range(NCORE):
        for i in range(TPC):
            t = k * TPC + i
            row = np.concatenate([ei[t, 0].astype(np.int64), np.arange(N)])
            col = np.concatenate([ei[t, 1].astype(np.int64), np.arange(N)])
            w = np.concatenate([ew[t], np.ones(N)])
            deg = np.bincount(col, weights=w, minlength=N)
            dis = 1.0 / np.sqrt(deg)
            norm = (dis[row] * w * dis[col]).astype(np.float32)
            order = np.argsort(col, kind="stable")
            per_ct[k][i] = (row[order], col[order], norm[order])

    templates, idx_all, slab_all = [], [[None] * TPC for _ in range(NCORE)], [[None] * TPC for _ in range(NCORE)]
    for i in range(TPC):
        counts = np.zeros((NCORE, N), np.int64)
        for k in range(NCORE):
            counts[k] = np.bincount(per_ct[k][i][1], minlength=N)

        chunks, c_lo, fill = [], 0, np.zeros(NCORE, np.int64)
        for c in range(N):
            if (c > c_lo and c % 128 == 0) or np.max(fill + counts[:, c]) > 128:
                chunks.append((c_lo, c))
                c_lo, fill = c, np.zeros(NCORE, np.int64)
            fill += counts[:, c]
            assert fill.max() <= 128, f"col {c} overflow"
        chunks.append((c_lo, N))
        chunks.append((N, NPAD))  # dummy: defines psum on pad cols

        fixed = [(lo, hi - lo, lo // 128) for (lo, hi) in chunks]
        nchunk = len(fixed)
        slabw = sum(c[1] for c in fixed)
        templates.append({"chunks": fixed, "nchunk": nchunk, "slabw": slabw})

        for k in range(NCORE):
            row, col, norm = per_ct[k][i]
            ptr, off = 0, 0
            idx_flat = np.full(nchunk * 128, DUMP, np.int16)
            slab = np.zeros((128, slabw), np.float32)
            for ck, (lo, ncols, blk) in enumerate(fixed):
                e0 = ptr
                ptr = int(np.searchsorted(col, lo + ncols, side="left"))
                ne = ptr - e0
                assert ne <= 128
                if ne:
                    idx_flat[ck * 128: ck * 128 + ne] = row[e0:ptr].astype(np.int16)
                    slab[np.arange(ne), off + (col[e0:ptr] - lo)] = norm[e0:ptr]
                off += ncols
            assert ptr == len(col)
            nidx = nchunk * 128
            wrapped = np.empty((16, nidx // 16), np.int16)
            wrapped[np.arange(nidx) % 16, np.arange(nidx) // 16] = idx_flat
            idx_all[k][i] = np.tile(wrapped, (8, 1))
            slab_all[k][i] = slab
    return templates, idx_all, slab_all


def make_sentinel(Wih, target=1.0e4):
    Wz = Wih[32:64, :]
    v, *_ = np.linalg.lstsq(Wz.astype(np.float64), np.full(32, target), rcond=None)
    return v.astype(np.float32)


# ---------------------------------------------------------------- gcn program

def build_gcn_program(templates):
    nc = bacc.Bacc()
    x_ext = nc.declare_dram_parameter("x", [TPC, F, NPAD], F32, isOutput=False)
    w1_ext = nc.declare_dram_parameter("w1", [F, H1], F32, isOutput=False)
    w2_ext = nc.declare_dram_parameter("w2", [H1, H1], F32, isOutput=False)
    b1_ext = nc.declare_dram_parameter("b1", [H1, 1], F32, isOutput=False)
    b2_ext = nc.declare_dram_parameter("b2", [H1, 1], F32, isOutput=False)
    idx_ext, slab_ext = [], []
    for i, tm in enumerate(templates):
        idx_ext.append(nc.declare_dram_parameter(
            f"idx{i}", [128, tm["nchunk"] * 8], I16, isOutput=False))
        slab_ext.append(nc.declare_dram_parameter(
            f"slab{i}", [128, tm["slabw"]], F32, isOutput=False))
    gcn_ext = nc.declare_dram_parameter("gcnT", [H1, TPC * NPAD], F32, isOutput=True)

    tbls = [nc.dram_tensor(f"tbl{j}", [NPAD, H1], F32) for j in range(4)]

    with tile.TileContext(nc) as tc:
        with (
            tc.tile_pool(name="cst", bufs=1) as cst,
            tc.tile_pool(name="per_t", bufs=2) as per_t,
            tc.tile_pool(name="acts", bufs=1) as acts,
            tc.tile_pool(name="stage", bufs=1) as stage,
            tc.tile_pool(name="msgs", bufs=2) as msgp,
            tc.tile_pool(name="ps_tf", bufs=2, space="PSUM") as ps_tf,
            tc.tile_pool(name="ps_tp", bufs=2, space="PSUM") as ps_tp,
            tc.tile_pool(name="ps_a", bufs=4, space="PSUM") as ps_a,
        ):
            w1_t = cst.tile([F, H1], F32)
            nc.sync.dma_start(out=w1_t[:], in_=w1_ext[:])
            w2_t = cst.tile([H1, H1], F32)
            nc.sync.dma_start(out=w2_t[:], in_=w2_ext[:])
            b1_t = cst.tile([H1, 1], F32)
            nc.sync.dma_start(out=b1_t[:], in_=b1_ext[:])
            b2_t = cst.tile([H1, 1], F32)
            nc.sync.dma_start(out=b2_t[:], in_=b2_ext[:])
            ident = cst.tile([H1, H1], F32)
            make_identity(nc, ident)

            for i, tm in enumerate(templates):
                nchunk, chunks = tm["nchunk"], tm["chunks"]
                idx_t = per_t.tile([128, nchunk * 8], I16, tag="idx")
                nc.sync.dma_start(out=idx_t[:], in_=idx_ext[i][:])
                slab_t = per_t.tile([128, tm["slabw"]], F32, tag="slab")
                nc.sync.dma_start(out=slab_t[:], in_=slab_ext[i][:])
                x_t = per_t.tile([F, NPAD], F32, tag="x")
                nc.sync.dma_start(out=x_t[:], in_=x_ext[i])

                def layer(src_t, w_t, b_t, out_name, tbl, out_sb):
                    # transform: pT = w.T @ src  [64, NPAD]
                    pT = acts.tile([H1, NPAD], F32, tag=f"pT")
                    for j in range(NPAD // 512 + 1):
                        lo = j * 512
                        hi = min(lo + 512, NPAD)
                        if lo >= hi:
                            break
                        pp = ps_tf.tile([H1, 512], F32, space="PSUM", tag="tf")
                        nc.tensor.matmul(out=pp[:, :hi - lo], lhsT=w_t[:],
                                         rhs=src_t[:, lo:hi], start=True, stop=True)
                        nc.vector.tensor_copy(out=pT[:, lo:hi], in_=pp[:, :hi - lo])
                    # transpose -> table (node-major) -> HBM
                    tbl_sb = stage.tile([128, NPAD // 128, H1], F32, tag="tbl")
                    for j in range(NPAD // 128):
                        tp = ps_tp.tile([128, H1], F32, space="PSUM", tag="tp")
                        nc.tensor.transpose(out=tp[:], in_=pT[:, j * 128:(j + 1) * 128],
                                            identity=ident[:])
                        nc.vector.tensor_copy(out=tbl_sb[:, j, :], in_=tp[:])
                    nc.sync.dma_start(
                        out=tbl[:].rearrange("(j p) w -> p j w", p=128), in_=tbl_sb[:])
                    # gather + slab matmuls + biased-relu evictions
                    npair = NPAD // 256  # psum tiles span 256 cols
                    pair_tiles = [None] * npair
                    pair_left = [0] * npair
                    for (lo, ncols, blk) in chunks:
                        pair_left[blk // 2] += 1
                    ngroup = (nchunk + GATHER_GROUP - 1) // GATHER_GROUP
                    off = 0
                    ck = 0
                    for g in range(ngroup):
                        gc = min(GATHER_GROUP, nchunk - g * GATHER_GROUP)
                        mbuf = msgp.tile([128, GATHER_GROUP, H1], F32, tag="m")
                        nc.gpsimd.dma_gather(
                            mbuf[:, :gc, :], tbl[:],
                            idx_t[:, g * GATHER_GROUP * 8: g * GATHER_GROUP * 8 + gc * 8],
                            gc * 128, gc * 128, H1,
                            single_packet=False,
                        )
                        for q in range(gc):
                            lo, ncols, blk = chunks[ck]
                            pr = blk // 2
                            if pair_tiles[pr] is None:
                                pair_tiles[pr] = ps_a.tile(
                                    [H1, 256], F32, space="PSUM", tag="A", name=f"A_{pr}")
                            po = lo - 256 * pr
                            nc.tensor.matmul(
                                out=pair_tiles[pr][:, po:po + ncols],
                                lhsT=mbuf[:, q, :], rhs=slab_t[:, off:off + ncols],
                                start=True, stop=True)
                            off += ncols
                            ck += 1
                            pair_left[pr] -= 1
                            if pair_left[pr] == 0:
                                lo2 = pr * 256
                                hi2 = min(lo2 + 256, NPAD)
                                nc.scalar.activation(
                                    out=out_sb[:, lo2:hi2], in_=pair_tiles[pr][:, :hi2 - lo2],
                                    func=mybir.ActivationFunctionType.Relu,
                                    bias=b_t[:])
                    assert ck == nchunk

                h1T = acts.tile([H1, NPAD], F32, tag="h1T")
                layer(x_t, w1_t, b1_t, "h1", tbls[i % 2], h1T)
                gcn_sb = acts.tile([H1, NPAD], F32, tag="gcn")
                layer(h1T, w2_t, b2_t, "gcn", tbls[2 + i % 2], gcn_sb)
                nc.sync.dma_start(out=gcn_ext[:, i * NPAD:(i + 1) * NPAD], in_=gcn_sb[:])
    nc.compile()
    return nc


# ---------------------------------------------------------------- gru program

def build_gru_program():
    nc = bacc.Bacc()
    # halo packed [128, 24576]: partitions 0-63 = halo timesteps 0..7,
    # partitions 64-127 = halo timesteps 8..14 (+ zero pad tail).
    HCOLS = TPC * NPAD  # 24576
    halo_ext = nc.declare_dram_parameter("halo", [128, HCOLS], F32, isOutput=False)
    # lhsA duplicated at partition bases 0/64; lhsB and wp at 0/32/64/96
    lhsA_ext = nc.declare_dram_parameter("lhsA", [128, 128], F32, isOutput=False)
    lhsB_ext = nc.declare_dram_parameter("lhsB", [128, 128], F32, isOutput=False)
    wp_ext = nc.declare_dram_parameter("wp", [128, 1], F32, isOutput=False)
    br_ext = nc.declare_dram_parameter("b_r", [H2, 1], F32, isOutput=False)
    bz_ext = nc.declare_dram_parameter("b_z", [H2, 1], F32, isOutput=False)
    bin_ext = nc.declare_dram_parameter("bih_n", [H2, 1], F32, isOutput=False)
    bhn_ext = nc.declare_dram_parameter("bhh_n", [H2, 1], F32, isOutput=False)
    bp_ext = nc.declare_dram_parameter("bp", [1, 1], F32, isOutput=False)
    preds_ext = nc.declare_dram_parameter("preds", [TPC, NPAD], F32, isOutput=True)

    QS = TPC * NPAD // 3  # 8192: hT strip width (bases 0/32/64 only)

    with tile.TileContext(nc) as tc:
        with (
            tc.tile_pool(name="cst", bufs=1) as cst,
            tc.tile_pool(name="big", bufs=1) as big,
            tc.tile_pool(name="work", bufs=2) as work,
            tc.tile_pool(name="ps", bufs=4, space="PSUM") as ps,
            tc.tile_pool(name="psp", bufs=2, space="PSUM") as psp,
        ):
            lhsA = cst.tile([128, 128], F32)
            nc.sync.dma_start(out=lhsA[:], in_=lhsA_ext[:])
            lhsB = cst.tile([128, 128], F32)
            nc.sync.dma_start(out=lhsB[:], in_=lhsB_ext[:])
            wp = cst.tile([128, 1], F32)
            nc.sync.dma_start(out=wp[:], in_=wp_ext[:])
            br = cst.tile([H2, 1], F32)
            nc.sync.dma_start(out=br[:], in_=br_ext[:])
            bz = cst.tile([H2, 1], F32)
            nc.sync.dma_start(out=bz[:], in_=bz_ext[:])
            bin_ = cst.tile([H2, 1], F32)
            nc.sync.dma_start(out=bin_[:], in_=bin_ext[:])
            bhn = cst.tile([H2, 1], F32)
            nc.sync.dma_start(out=bhn[:], in_=bhn_ext[:])
            bp = cst.tile([1, 1], F32)
            nc.sync.dma_start(out=bp[:], in_=bp_ext[:])

            halo = big.tile([128, HCOLS], F32)
            nc.sync.dma_start(out=halo[:], in_=halo_ext[:])
            hT = big.tile([96, QS], F32)
            nc.vector.memset(hT[:], 0.0)

            def h_strip(g0):  # global gru col -> (partition base, local col)
                return 32 * (g0 // QS), g0 % QS

            for s in range(WWIN):
                for j in range(NSLICE):
                    g0 = j * 512
                    h0 = s * NPAD + g0     # halo global col
                    ha_b, ha_c = (0, h0) if h0 < HCOLS else (64, h0 - HCOLS)
                    hb, hc = h_strip(g0)
                    pp = ps.tile([128, 512], F32, space="PSUM", tag="g")
                    nc.tensor.matmul(out=pp[:], lhsT=lhsA[ha_b:ha_b + 64, :],
                                     rhs=halo[ha_b:ha_b + 64, ha_c:ha_c + 512],
                                     start=True, stop=False)
                    nc.tensor.matmul(out=pp[:], lhsT=lhsB[hb:hb + 32, :],
                                     rhs=hT[hb:hb + 32, hc:hc + 512],
                                     start=False, stop=True)
                    r_t = work.tile([H2, 512], F32, tag="r")
                    nc.scalar.activation(out=r_t[:], in_=pp[0:32, :],
                                         func=mybir.ActivationFunctionType.Sigmoid,
                                         bias=br[:])
                    z_t = work.tile([H2, 512], F32, tag="z")
                    nc.scalar.activation(out=z_t[:], in_=pp[32:64, :],
                                         func=mybir.ActivationFunctionType.Sigmoid,
                                         bias=bz[:])
                    hnb = work.tile([H2, 512], F32, tag="hnb")
                    nc.scalar.activation(out=hnb[:], in_=pp[96:128, :],
                                         func=mybir.ActivationFunctionType.Identity,
                                         bias=bhn[:])
                    rhn = work.tile([H2, 512], F32, tag="rhn")
                    nc.vector.tensor_tensor(out=rhn[:], in0=r_t[:], in1=hnb[:],
                                            op=mybir.AluOpType.mult)
                    tt = work.tile([H2, 512], F32, tag="tt")
                    nc.vector.tensor_tensor(out=tt[:], in0=pp[64:96, :], in1=rhn[:],
                                            op=mybir.AluOpType.add)
                    n_t = work.tile([H2, 512], F32, tag="n")
                    nc.scalar.activation(out=n_t[:], in_=tt[:],
                                         func=mybir.ActivationFunctionType.Tanh,
                                         bias=bin_[:])
                    hcur = work.tile([H2, 512], F32, tag="hcur")
                    nc.gpsimd.tensor_copy(out=hcur[:], in_=hT[hb:hb + 32, hc:hc + 512])
                    hmn = work.tile([H2, 512], F32, tag="hmn")
                    nc.vector.tensor_tensor(out=hmn[:], in0=hcur[:],
                                            in1=n_t[:], op=mybir.AluOpType.subtract)
                    zh = work.tile([H2, 512], F32, tag="zh")
                    nc.vector.tensor_tensor(out=zh[:], in0=z_t[:], in1=hmn[:],
                                            op=mybir.AluOpType.mult)
                    nc.vector.tensor_tensor(out=hT[hb:hb + 32, hc:hc + 512],
                                            in0=n_t[:], in1=zh[:],
                                            op=mybir.AluOpType.add)

            SPW = NPAD // 512  # slices per window (6)
            for w in range(TPC):
                pw = work.tile([1, NPAD], F32, tag="pw")
                for jj in range(SPW):
                    j = w * SPW + jj
                    hb, hc = h_strip(j * 512)
                    pq = psp.tile([1, 512], F32, space="PSUM", tag="p")
                    nc.tensor.matmul(out=pq[:], lhsT=wp[hb:hb + 32, :],
                                     rhs=hT[hb:hb + 32, hc:hc + 512],
                                     start=True, stop=True)
                    nc.scalar.activation(out=pw[0:1, jj * 512:(jj + 1) * 512], in_=pq[:],
                                         func=mybir.ActivationFunctionType.Identity,
                                         bias=bp[:])
                nc.sync.dma_start(out=preds_ext[w:w + 1, :], in_=pw[:])
    nc.compile()
    return nc


# ---------------------------------------------------------------- entry point

def kernel(**inputs) -> np.ndarray:
    x = np.asarray(inputs["x"], np.float32)
    ei = np.asarray(inputs["edge_index"])
    ew = np.asarray(inputs["edge_weight"], np.float32)
    W1 = np.asarray(inputs["W1"], np.float32)
    b1 = np.asarray(inputs["b1"], np.float32)
    W2 = np.asarray(inputs["W2"], np.float32)
    b2 = np.asarray(inputs["b2"], np.float32)
    Wih = np.asarray(inputs["Wih"], np.float32)
    Whh = np.asarray(inputs["Whh"], np.float32)
    bih = np.asarray(inputs["bih"], np.float32)
    bhh = np.asarray(inputs["bhh"], np.float32)
    Wp = np.asarray(inputs["Wp"], np.float32)
    bp = np.asarray(inputs["bp"], np.float32)
    assert int(inputs["consider_time_steps"]) == WWIN

    templates, idx_all, slab_all = build_templates_and_pack(ei, ew)

    sig = tuple((tm["nchunk"], tm["slabw"]) for tm in templates)
    if ("gcn", sig) not in _PROG_CACHE:
        _PROG_CACHE[("gcn", sig)] = build_gcn_program(templates)
    nc_gcn = _PROG_CACHE[("gcn", sig)]

    xpad = np.zeros((T, F, NPAD), np.float32)
    xpad[:, :, :N] = x
    in_maps = []
    for k in range(NCORE):
        m = {
            "x": xpad[k * TPC:(k + 1) * TPC],
            "w1": W1, "w2": W2, "b1": b1[:, None], "b2": b2[:, None],
        }
        for i in range(TPC):
            m[f"idx{i}"] = idx_all[k][i]
            m[f"slab{i}"] = slab_all[k][i]
        in_maps.append(m)
    res = run_bass_kernel_spmd(nc_gcn, in_maps, core_ids=list(range(NCORE)))
    gcnT_all = np.concatenate([res.results[k]["gcnT"] for k in range(NCORE)], axis=1)

    if "gru" not in _PROG_CACHE:
        _PROG_CACHE["gru"] = build_gru_program()
    nc_gru = _PROG_CACHE["gru"]

    sent = make_sentinel(Wih)
    lhsA1 = np.zeros((H1, 128), np.float32)
    lhsA1[:, 0:64] = Wih.T[:, 0:64]
    lhsA1[:, 64:96] = Wih.T[:, 64:96]
    lhsB1 = np.zeros((H2, 128), np.float32)
    lhsB1[:, 0:64] = Whh.T[:, 0:64]
    lhsB1[:, 96:128] = Whh.T[:, 64:96]
    lhsA = np.zeros((128, 128), np.float32)
    lhsA[0:64] = lhsA1
    lhsA[64:128] = lhsA1
    lhsB = np.zeros((128, 128), np.float32)
    wp = np.zeros((128, 1), np.float32)
    for q in range(3):
        lhsB[32 * q:32 * q + 32] = lhsB1
        wp[32 * q:32 * q + 32] = Wp.astype(np.float32)

    HCOLS = TPC * NPAD
    in_maps2 = []
    for k in range(NCORE):
        halo_flat = np.empty((H1, HALO * NPAD), np.float32)
        t0 = k * TPC - (WWIN - 1)
        for j, t in enumerate(range(t0, k * TPC + TPC)):
            if t < 0:
                halo_flat[:, j * NPAD:(j + 1) * NPAD] = sent[:, None]
            else:
                halo_flat[:, j * NPAD:(j + 1) * NPAD] = gcnT_all[:, t * NPAD:(t + 1) * NPAD]
        halo = np.zeros((128, HCOLS), np.float32)
        halo[0:64, :] = halo_flat[:, :HCOLS]
        halo[64:128, :HALO * NPAD - HCOLS] = halo_flat[:, HCOLS:]
        in_maps2.append({
            "halo": halo, "lhsA": lhsA, "lhsB": lhsB, "wp": wp,
            "b_r": (bih[0:32] + bhh[0:32]).astype(np.float32)[:, None],
            "b_z": (bih[32:64] + bhh[32:64]).astype(np.float32)[:, None],
            "bih_n": bih[64:96].astype(np.float32)[:, None],
            "bhh_n": bhh[64:96].astype(np.float32)[:, None],
            "bp": np.asarray(bp, np.float32).reshape(1, 1),
        })
    res2 = run_bass_kernel_spmd(nc_gru, in_maps2, core_ids=list(range(NCORE)))

    preds = np.zeros((T, N), np.float32)
    for k in range(NCORE):
        p = res2.results[k]["preds"][:, :N]
        preds[k * TPC:(k + 1) * TPC] = p
    return preds
